# revision 41
# baseline (speedup 1.0000x reference)
"""Trainium2 Bass kernel for a dense transformer block.

Data-parallel over batch B=8 across 8 NeuronCores (one batch element per
core, weights replicated, no collectives).

Per core (x_b is [T=1024, C=1024] fp32):
  h  = LN1(x);  per-head q,k,v = h @ Wq/Wk/Wv;  S = q k^T / 8 with the
  "staircase" mask == block-causal at 64 granularity;  out = softmax(S) v
  x2 = x + cat(out) @ Wo + bo;  y = x2 + relu(LN2(x2) @ W1 + b1) @ W2 + b2

v15 design notes (on top of v13):
  - Q/K projections in fp8e4 DoubleRow: hT stored fp8 channel-major
    [128, NCH, T]; contraction pairs = channel-chunk pairs (2g, 2g+1) so
    hT[:, 2g:2g+2, :] is directly the DoubleRow moving AP; wq/wk
    pre-paired host-side to match.  V stays bf16 (mixed fp8xbf16 matmul)
    -- v errors pass straight through attention, q/k errors are damped
    by softmax normalization, so this is where the error budget goes.
  - attention A@V in fp8e4 DoubleRow pairing *key-block* j with j+1:
    exp(S) written fp8 into [128, 2, 1024] tiles (j-depth), V stationary
    [128, 2, 65]; ragged mask edges zero-filled (gpsimd, two-head strided
    memsets) so pairs stay legal. exp biased by -2.5 so fp8e4 never
    saturates at 240 (softmax ratio is bias-invariant).
  - attention is ACT(exp)-bound: LN normalizes moved to ACT Identity
    (per-partition scale/bias APs) in phases 1/3 where ACT idles, and the
    hT copy runs on ACT too, keeping DVE for recip/divide/bias.
  - S stays bf16 (fp8 S noise is error-prohibitive on top of qk8).
  - LN affines folded into Wq/Wk/Wv/W1 host-side; per-channel biases as
    per-partition adds on q/k, K=1 ones-row matmuls for v/bo/b2.
  - FFN2: FP8G=12 k-groups in fp8-DoubleRow (rest bf16), all 8 PSUM banks
    as full-T accumulators per column half so W2 streams exactly once.
    More fp8 (FFN1, Wo, S) busts the 2e-2 gate: HW rel err is 1.955e-2.
  - softmax denominator from the ones column of V, reciprocal + gpsimd
    partition_broadcast, divide-multiply on DVE.
  - warm-up matmuls flip the PE HAM clock gate while x streams in.
"""

import os

import numpy as np

import concourse.bass as bass
import concourse.mybir as mybir
import concourse.tile as tile
from concourse import bacc
from concourse.masks import make_identity
from concourse.bass_utils import run_bass_kernel_spmd

T, C, H, HS = 1024, 1024, 16, 64
NT = T // 128
NCH = C // 128
NPAIR = H // 2
FF = 4 * C
NG = FF // 128
FP8G = 12            # FFN2 k-groups in fp8-DoubleRow (rest bf16)
EPS = 1e-5
EXP_BIAS = -2.5      # exp(S/8 - 2.5): keeps fp8e4 exp() under 240
F32 = mybir.dt.float32
BF16 = mybir.dt.bfloat16
F8 = mybir.dt.float8e4
DR = mybir.MatmulPerfMode.DoubleRow


def _ln_stats(nc, pool, x_ap, eps_tile):
    """mean/rstd of [128,1024] fp32 tile -> (negmr, rstd) such that the
    normalize is a single ACT Copy: h = x*rstd + negmr."""
    stats = pool.tile([128, 2, 6], F32, tag="ln_stats", name="ln_stats")
    mv = pool.tile([128, 2], F32, tag="ln_mv", name="ln_mv")
    xr = x_ap.rearrange("p (s f) -> p s f", s=2)
    for s in range(2):
        nc.vector.bn_stats(out=stats[:, s, :], in_=xr[:, s, :])
    nc.vector.bn_aggr(out=mv, in_=stats)
    rstd = pool.tile([128, 1], F32, tag="ln_rstd", name="ln_rstd")
    nc.scalar.activation(
        out=rstd, in_=mv[:, 1:2],
        func=mybir.ActivationFunctionType.Sqrt,
        bias=eps_tile, scale=1.0,
    )
    nc.vector.reciprocal(out=rstd, in_=rstd)
    negmr = pool.tile([128, 1], F32, tag="ln_negmr", name="ln_negmr")
    nc.vector.scalar_tensor_tensor(
        out=negmr, in0=mv[:, 0:1], scalar=-1.0, in1=rstd,
        op0=mybir.AluOpType.mult, op1=mybir.AluOpType.mult)
    return negmr, rstd


def build_program():
    nc = bacc.Bacc("TRN2", target_bir_lowering=False, debug=False, num_devices=8)

    x_d = nc.dram_tensor("x", [T, C], F32, kind="ExternalInput").ap()
    wqk_d = nc.dram_tensor("wqk", [NPAIR, 128, 2048], F8,
                           kind="ExternalInput").ap()
    wv_d = nc.dram_tensor("wv", [128, NCH * C], BF16,
                          kind="ExternalInput").ap()   # [128, 8192] packed
    wo_d = nc.dram_tensor("wo", [C, C], BF16, kind="ExternalInput").ap()
    w1_d = nc.dram_tensor("w1", [NG, 128, C], BF16, kind="ExternalInput").ap()
    if FP8G:
        w2_d = nc.dram_tensor("w2", [FP8G // 2, 128, 2, C], F8,
                              kind="ExternalInput").ap()
    w2b_d = nc.dram_tensor("w2b", [(NG - FP8G) * 128, C], BF16,
                           kind="ExternalInput").ap()
    bq_d = nc.dram_tensor("bq", [C], F32, kind="ExternalInput").ap()
    bk_d = nc.dram_tensor("bk", [C], F32, kind="ExternalInput").ap()
    bv_d = nc.dram_tensor("bv", [C], BF16, kind="ExternalInput").ap()
    bo_d = nc.dram_tensor("bo", [C], BF16, kind="ExternalInput").ap()
    b1_d = nc.dram_tensor("b1", [FF], F32, kind="ExternalInput").ap()
    b2_d = nc.dram_tensor("b2", [C], F8, kind="ExternalInput").ap()
    y_d = nc.dram_tensor("y", [T, C], F32, kind="ExternalOutput").ap()

    reps = int(os.environ.get("KERNEL_REPS", "1"))
    with tile.TileContext(nc) as tc:
        for r in range(reps):
            _emit(nc, tc, x_d, wqk_d, wv_d, wo_d, w1_d,
                  w2_d if FP8G else None, w2b_d,
                  bq_d, bk_d, bv_d, bo_d, b1_d, b2_d, y_d,
                  warmup=(r == 0))
    nc.compile()
    return nc


def _emit(nc, tc, x_d, wqk_d, wv_d, wo_d, w1_d, w2_d, w2b_d,
          bq_d, bk_d, bv_d, bo_d, b1_d, b2_d, y_d, warmup=False):
    singles = tc.alloc_tile_pool(name="singles", bufs=1)
    # One global PSUM pool; tags: sc01 (2x2 banks), av (2x1), big (2x1).
    ps_pool = tc.alloc_tile_pool(name="ps", bufs=1, space="PSUM")
    # PE warm-up first: depends only on one DVE memset, so the HAM clock
    # gate flips to 2.4GHz while everything else (x DMA, LN1) is starting.
    if warmup:
        junk = singles.tile([128, 512], BF16, name="junk")
        nc.vector.memset(junk, 0.0)
        for _ in range(48):
            ps = ps_pool.tile(
                [128, 512], F32, tag="av", bufs=2, name="ps_warm")
            nc.tensor.matmul(ps, junk[:, 0:128], junk, start=True, stop=True)
    identf = singles.tile([128, 128], F32, name="identf")
    make_identity(nc, identf)
    ident = singles.tile([128, 128], BF16, name="ident")
    nc.vector.tensor_copy(out=ident, in_=identf)
    eps_tile = singles.tile([128, 1], F32, name="eps")
    nc.vector.memset(eps_tile, EPS)
    ebias_tile = singles.tile([128, 1], F32, name="ebias")
    nc.vector.memset(ebias_tile, EXP_BIAS)
    ones_r = singles.tile([1, 128], BF16, name="ones_r")
    nc.vector.memset(ones_r, 1.0)
    ones_f8 = singles.tile([1, 128], F8, name="ones_f8")
    nc.vector.memset(ones_f8, 1.0)
    b1_sb = singles.tile([128, NG], F32, name="b1_sb")
    nc.sync.dma_start(out=b1_sb, in_=b1_d.rearrange("(g p) -> p g", p=128))
    bq_sb = singles.tile([128, NPAIR], F32, name="bq_sb")
    nc.sync.dma_start(out=bq_sb, in_=bq_d.rearrange("(g p) -> p g", p=128))
    bk_sb = singles.tile([128, NPAIR], F32, name="bk_sb")
    nc.sync.dma_start(out=bk_sb, in_=bk_d.rearrange("(g p) -> p g", p=128))
    bv_sb = singles.tile([1, C], BF16, name="bv_sb")
    nc.sync.dma_start(out=bv_sb, in_=bv_d.unsqueeze(0))
    bo_sb = singles.tile([1, C], BF16, name="bo_sb")
    nc.sync.dma_start(out=bo_sb, in_=bo_d.unsqueeze(0))
    b2_sb = singles.tile([1, C], F8, name="b2_sb")
    nc.sync.dma_start(out=b2_sb, in_=b2_d.unsqueeze(0))

    ln_pool = tc.alloc_tile_pool(name="ln", bufs=3)

    def big():
        return ps_pool.tile([128, 512], F32, tag="big", bufs=2, name="ps_big")

    hT_pool = tc.alloc_tile_pool(name="hTp", bufs=1)
    hT = hT_pool.tile([128, NCH, T], F8, name="hT")
    x2_pool = tc.alloc_tile_pool(name="x2p", bufs=1)
    x2 = x2_pool.tile([128, NT, C], F32, name="x2")
    w1_pool = tc.alloc_tile_pool(name="w1p", bufs=4)
    wo_pool = tc.alloc_tile_pool(name="wop", bufs=1)
    wo_t = wo_pool.tile([128, NCH, C], BF16, name="wo_t")
    w_pool = tc.alloc_tile_pool(name="wqk", bufs=3)
    v_pool = tc.alloc_tile_pool(name="vAp", bufs=1)
    v_all = v_pool.tile([128, NT, H * 65], F8, name="v_all")
    for hh in range(H):
        nc.gpsimd.memset(v_all[:, :, 65 * hh + 64:65 * hh + 65], 1.0)

    h2T_pool = tc.alloc_tile_pool(name="h2Tp", bufs=1, side="right")
    h2T = h2T_pool.tile([128, NCH, T], BF16, name="h2T")
    outT_pool = tc.alloc_tile_pool(name="outTp", bufs=1, side="right")
    outT = outT_pool.tile([128, NPAIR, T], BF16, name="outT")

    wqk_tiles = {}

    def load_pair(pp):
        wqk_t = w_pool.tile([128, 2048], F8, tag="wqk", name="wqk_t")
        nc.sync.dma_start(out=wqk_t, in_=wqk_d[pp])
        wqk_tiles[pp] = wqk_t

    # ---- Phase 1: LN1 -> hT (channel-major, fp8) + V (fp8 DoubleRow) ----
    v_view = v_all.rearrange("p i (h d) -> p i h d", h=H)
    with tc.tile_pool(name="h", bufs=3) as h_pool, \
         tc.tile_pool(name="xin1", bufs=4) as x_pool, \
         tc.tile_pool(name="wvg", bufs=1) as wv_pool:
        wv_t = wv_pool.tile([128, NCH * 1024], BF16, name="wv_t")

        def load_x(ii):
            x_t = x_pool.tile([128, C], F32, tag="x", name="x_t")
            for s in range(2):
                nc.sync.dma_start(
                    out=x_t[:, s * 512:(s + 1) * 512],
                    in_=x_d[ii * 128:(ii + 1) * 128, s * 512:(s + 1) * 512])
            return x_t

        x_ts = {0: load_x(0), 1: load_x(1)}
        for grp in range(2):
            nc.sync.dma_start(
                out=wv_t[:, grp * 4096:(grp + 1) * 4096],
                in_=wv_d[:, grp * 4096:(grp + 1) * 4096])
        load_pair(0)
        load_pair(1)
        stats = {0: _ln_stats(nc, ln_pool, x_ts[0], eps_tile)}
        for i in range(NT):
            if i + 2 < NT:
                x_ts[i + 2] = load_x(i + 2)
            negmr, rstd = stats.pop(i)
            x_t = x_ts.pop(i)
            h_t = h_pool.tile([128, C], BF16, tag="h", name="h_t")
            nc.scalar.activation(
                out=h_t, in_=x_t,
                func=mybir.ActivationFunctionType.Identity,
                scale=rstd, bias=negmr)
            ps_tr = ps_pool.tile([128, 1024], BF16, tag="av",
                                 bufs=2, name="ps_tr")
            for j in range(NCH):
                nc.tensor.transpose(
                    ps_tr[:, j * 128:(j + 1) * 128],
                    h_t[:, j * 128:(j + 1) * 128], ident)
            nc.scalar.activation(
                out=hT[:, :, i * 128:(i + 1) * 128],
                in_=ps_tr.rearrange("p (j t) -> p j t", j=NCH),
                func=mybir.ActivationFunctionType.Copy)
            if i + 1 < NT:
                stats[i + 1] = _ln_stats(nc, ln_pool, x_ts[i + 1], eps_tile)
            for grp in range(2):
                ps_v = big()
                nc.tensor.matmul(
                    ps_v, ones_r, bv_sb[0:1, grp * 512:(grp + 1) * 512],
                    start=True, stop=False, skip_group_check=True)
                for j in range(NCH):
                    nc.tensor.matmul(
                        ps_v, hT[:, j, i * 128:(i + 1) * 128],
                        wv_t[:, j * 1024 + grp * 512:j * 1024 + (grp + 1) * 512],
                        skip_group_check=True,
                        start=False, stop=(j == NCH - 1))
                nc.vector.tensor_copy(
                    out=v_view[:, i, grp * 8:(grp + 1) * 8, 0:64],
                    in_=ps_v.rearrange("p (h d) -> p h d", h=8))

    # hoist wo loads: DMA queue is idle during attention
    for ch in range(NCH):
        nc.sync.dma_start(
            out=wo_t[:, ch, :], in_=wo_d[ch * 128:(ch + 1) * 128, :])

    # ---- Phase 2: per head-pair QK + attention (V stationary) ----
    with tc.tile_pool(name="qk", bufs=2) as qk_pool, \
         tc.tile_pool(name="expS", bufs=14) as e_pool, \
         tc.tile_pool(name="rec", bufs=4) as r_pool, \
         tc.tile_pool(name="rbc", bufs=4) as rb_pool:
        def qkproj(p):
            wqk_t = wqk_tiles.pop(p)
            qT = qk_pool.tile([128, T], BF16, tag="qT", name="qT")
            kT = qk_pool.tile([128, T], BF16, tag="kT", name="kT")
            for di, (dst, bias) in enumerate(((qT, bq_sb), (kT, bk_sb))):
                for half in range(2):
                    ps = big()
                    for g in range(NCH // 2):
                        nc.tensor.matmul(
                            ps,
                            wqk_t[:, di * 1024 + g * 256:
                                  di * 1024 + (g + 1) * 256].rearrange(
                                      "p (j m) -> p j m", j=2),
                            hT[:, 2 * g:2 * g + 2,
                               half * 512:(half + 1) * 512],
                            perf_mode=DR,
                            start=(g == 0), stop=(g == NCH // 2 - 1))
                    nc.vector.tensor_scalar(
                        out=dst[:, half * 512:(half + 1) * 512], in0=ps,
                        scalar1=bias[:, p:p + 1], scalar2=None,
                        op0=mybir.AluOpType.add)
            return qT, kT

        def emit_s(qT, kT, th):
            t0 = th * 512
            njt = (th + 1) * 4
            eT = [None] * (njt // 2)
            for jp in range(njt // 2):
                et = e_pool.tile([128, 2, 1024], F8, tag="e", name="eS_t")
                c0e = max(0, (2 * jp) * 128 - t0)
                c0o = max(0, (2 * jp + 1) * 128 - t0)
                if c0o > c0e:
                    # odd block masked where even block isn't: zero so
                    # the DoubleRow pair contributes nothing there
                    # (both heads in one strided memset, on gpsimd)
                    nc.gpsimd.memset(
                        et[:, 1, :].rearrange("p (h q) -> p h q", h=2)
                        [:, :, c0e:c0o], 0.0)
                for dj in range(2):
                    j = 2 * jp + dj
                    c0 = max(0, j * 128 - t0)
                    ps = ps_pool.tile([128, 1024], F32, tag="s2",
                                      bufs=2, name="ps_sc")
                    for hh in range(2):
                        hsl = slice(hh * 64, (hh + 1) * 64)
                        nc.tensor.matmul(
                            ps[:, hh * 512 + c0:(hh + 1) * 512],
                            kT[hsl, j * 128:(j + 1) * 128],
                            qT[hsl, t0 + c0:t0 + 512],
                            start=True, stop=True,
                            tile_position=(hh * 64, 0))
                    pv = ps.rearrange("p (h q) -> p h q", h=2)
                    ev = et[:, dj, :].rearrange("p (h q) -> p h q", h=2)
                    nc.scalar.activation(
                        out=ev[:, :, c0:512], in_=pv[:, :, c0:512],
                        func=mybir.ActivationFunctionType.Exp,
                        scale=float(HS) ** -0.5, bias=ebias_tile)
                    if j * 128 >= t0:
                        nc.gpsimd.memset(
                            et[64:128, dj, :].rearrange(
                                "p (h q) -> p h q", h=2)
                            [:, :, c0:c0 + 64], 0.0)
                eT[jp] = et
            return eT

        def emit_av(p, th, eT):
            t0 = th * 512
            njt = (th + 1) * 4
            for hh in range(2):
                head = 2 * p + hh
                ps_av = ps_pool.tile([128, 512], F32, tag="av", bufs=2,
                                     name="ps_av")
                for jp in range(njt // 2):
                    c0 = max(0, (2 * jp) * 128 - t0)
                    nc.tensor.matmul(
                        ps_av[0:65, c0:512],
                        v_all[:, 2 * jp:2 * jp + 2,
                              65 * head:65 * head + 65],
                        eT[jp][:, :, hh * 512 + c0:(hh + 1) * 512],
                        perf_mode=DR,
                        start=(jp == 0), stop=(jp == njt // 2 - 1))
                rc = r_pool.tile([128, 512], F32, tag="rc", name="rc")
                nc.vector.reciprocal(out=rc[0:1, :], in_=ps_av[64:65, :])
                rb = rb_pool.tile([128, 512], F32, tag="rb", name="rb")
                nc.gpsimd.partition_broadcast(rb[0:64, :], rc[0:1, :])
                nc.vector.tensor_tensor(
                    out=outT[64 * hh:64 * (hh + 1), p, t0:t0 + 512],
                    in0=ps_av[0:64, :], in1=rb[0:64, :],
                    op=mybir.AluOpType.mult)

        # software pipeline: AV of pair p-1 sits between S(th0) and S(th1)
        # of pair p, and pair p+1's projections follow S(th1), so the ACT
        # exp() queue never starves the PE (and vice versa).
        qk_cur = qkproj(0)
        pend = None
        for p in range(NPAIR):
            if p + 2 < NPAIR:
                load_pair(p + 2)
            qT, kT = qk_cur
            eTs = {0: emit_s(qT, kT, 0)}
            if pend is not None:
                for th in range(2):
                    emit_av(pend[0], th, pend[1][th])
            eTs[1] = emit_s(qT, kT, 1)
            if p + 1 < NPAIR:
                qk_cur = qkproj(p + 1)
            pend = (p, eTs)
        for th in range(2):
            emit_av(pend[0], th, pend[1][th])
    v_pool.release()
    w_pool.release()

    # hoist the first FFN1 weight tiles
    w1_tiles = {}

    def load_w1(gg):
        w1_t = w1_pool.tile([128, C], BF16, tag="w1", name="w1_t")
        nc.sync.dma_start(out=w1_t, in_=w1_d[gg])
        w1_tiles[gg] = w1_t

    for gg in range(3):
        load_w1(gg)

    # ---- Phase 3+4: projection + residual (+bo) + LN2 -> h2T ----
    # Skewed: transposes of tile i-1 are emitted after tile i's proj
    # matmuls so the PE never waits on the DVE/ACT LN2 chain.
    with tc.tile_pool(name="xin2", bufs=4) as x_pool:

        def load_x2(ii):
            x_t = x_pool.tile([128, C], F32, tag="x", name="x_t2")
            for s in range(2):
                nc.sync.dma_start(
                    out=x_t[:, s * 512:(s + 1) * 512],
                    in_=x_d[ii * 128:(ii + 1) * 128, s * 512:(s + 1) * 512])
            return x_t

        x_ts2 = {0: load_x2(0), 1: load_x2(1)}
        h_ts = {}

        def emit_tr2(ii):
            h_t = h_ts.pop(ii)
            ps_tr = ps_pool.tile([128, 1024], BF16, tag="s2",
                                 bufs=2, name="ps_tr2")
            for j in range(NCH):
                nc.tensor.transpose(
                    ps_tr[:, j * 128:(j + 1) * 128],
                    h_t[:, j * 128:(j + 1) * 128], ident)
            nc.vector.tensor_copy(
                out=h2T[:, :, ii * 128:(ii + 1) * 128],
                in_=ps_tr.rearrange("p (j t) -> p j t", j=NCH))

        for i in range(NT):
            if i + 2 < NT:
                x_ts2[i + 2] = load_x2(i + 2)
            x_t = x_ts2.pop(i)
            for half in range(2):
                psh = ps_pool.tile([128, 512], F32, tag="av", bufs=2,
                                   name="ps_pr")
                nc.tensor.matmul(
                    psh, ones_r, bo_sb[0:1, half * 512:(half + 1) * 512],
                    start=True, stop=False)
                for ch in range(NCH):
                    nc.tensor.matmul(
                        psh, outT[:, ch, i * 128:(i + 1) * 128],
                        wo_t[:, ch, half * 512:(half + 1) * 512],
                        start=False, stop=(ch == NCH - 1))
                nc.vector.tensor_add(
                    out=x2[:, i, half * 512:(half + 1) * 512],
                    in0=psh, in1=x_t[:, half * 512:(half + 1) * 512])
            negmr, rstd = _ln_stats(nc, ln_pool, x2[:, i, :], eps_tile)
            h_t = x_pool.tile([128, C], BF16, tag="h2", name="h2_t")
            nc.scalar.activation(
                out=h_t, in_=x2[:, i, :],
                func=mybir.ActivationFunctionType.Identity,
                scale=rstd, bias=negmr)
            h_ts[i] = h_t
            if i >= 1:
                emit_tr2(i - 1)
        emit_tr2(NT - 1)
    outT_pool.release()
    wo_pool.release()

    # ---- Phase 5: FFN. W1 streamed once into full-T uT; W2 in 2 passes ----
    with tc.tile_pool(name="w2", bufs=8) as w2_pool, \
         tc.tile_pool(name="uTp", bufs=1) as uT_pool, \
         tc.tile_pool(name="yout", bufs=4) as out_pool:
        if FP8G:
            uT8 = uT_pool.tile([128, FP8G, T], F8, name="uT8")
        uTb = uT_pool.tile([128, NG - FP8G, T], BF16, name="uTb")
        for g in range(NG):
            if g + 3 < NG:
                load_w1(g + 3)
            w1_t = w1_tiles.pop(g)
            for th in range(2):
                ps = big()
                for j in range(NCH):
                    nc.tensor.matmul(
                        ps, w1_t[:, j * 128:(j + 1) * 128],
                        h2T[:, j, th * 512:(th + 1) * 512],
                        start=(j == 0), stop=(j == NCH - 1))
                udst = (uT8[:, g, :] if g < FP8G
                        else uTb[:, g - FP8G, :])
                nc.vector.tensor_scalar(
                    out=udst[:, th * 512:(th + 1) * 512],
                    in0=ps,
                    scalar1=b1_sb[:, g:g + 1], scalar2=0.0,
                    op0=mybir.AluOpType.add, op1=mybir.AluOpType.max)
        # FFN2: all 8 PSUM banks as full-T accumulators per column half,
        # so W2 streams exactly once.
        for chh in range(2):
            hsl = slice(chh * 512, (chh + 1) * 512)
            ps_w = [ps_pool.tile([128, 1024], F32, tag="s2",
                                 bufs=2, name=f"ps_w{iw}")
                    for iw in range(2)]
            ps_f = [ps_w[iw // 2][:, (iw % 2) * 512:(iw % 2 + 1) * 512]
                    for iw in range(4)]
            ps_f += [ps_pool.tile([128, 512], F32, tag="av", bufs=2,
                                  name=f"ps_a{iw}") for iw in range(2)]
            ps_f += [big(), big()]
            for it in range(8):
                nc.tensor.matmul(
                    ps_f[it], ones_f8, b2_sb[0:1, hsl],
                    start=True, stop=False, skip_group_check=True)
            for k2 in range(FP8G // 2):
                w2_t = w2_pool.tile([128, 2, 512], F8,
                                    tag="w28", name="w2_t8")
                nc.sync.dma_start(out=w2_t, in_=w2_d[k2][:, :, hsl])
                for it in range(8):
                    nc.tensor.matmul(
                        ps_f[it],
                        uT8[:, 2 * k2:2 * k2 + 2, it * 128:(it + 1) * 128],
                        w2_t,
                        perf_mode=DR, skip_group_check=True,
                        start=False, stop=False)
            for kb in range(NG - FP8G):
                w2_t = w2_pool.tile([128, 512], BF16, tag="w2b", name="w2_tb")
                nc.sync.dma_start(
                    out=w2_t, in_=w2b_d[kb * 128:(kb + 1) * 128, hsl])
                for it in range(8):
                    nc.tensor.matmul(
                        ps_f[it],
                        uTb[:, kb, it * 128:(it + 1) * 128],
                        w2_t, skip_group_check=True,
                        start=False, stop=(kb == NG - FP8G - 1))
            for it in range(8):
                o_t = out_pool.tile([128, 512], F32, tag="y", name="y_t")
                nc.vector.scalar_tensor_tensor(
                    out=o_t, in0=ps_f[it], scalar=1.0 / 16.0,
                    in1=x2[:, it, hsl],
                    op0=mybir.AluOpType.mult, op1=mybir.AluOpType.add)
                nc.sync.dma_start(
                    out=y_d[it * 128:(it + 1) * 128, hsl], in_=o_t)
    h2T_pool.release()
    w1_pool.release()
    x2_pool.release()
    hT_pool.release()
    ps_pool.release()
    ln_pool.release()
    singles.release()


_NC_CACHE = {}


def _get_program():
    if "nc" not in _NC_CACHE:
        _NC_CACHE["nc"] = build_program()
    return _NC_CACHE["nc"]


def _prep_inputs(x, Wq, Wk, Wv, Wo, bo, ln1_g, ln1_b, ln2_g, ln2_b, W1, b1, W2, b2):
    import ml_dtypes
    BF = ml_dtypes.bfloat16
    F8np = ml_dtypes.float8_e4m3
    f = lambda a: np.ascontiguousarray(np.asarray(a, dtype=np.float32))
    bf = lambda a: np.ascontiguousarray(np.asarray(a, np.float32).astype(BF))
    f8 = lambda a: np.ascontiguousarray(np.asarray(a, np.float32).astype(F8np))
    Wq, Wk, Wv = (np.asarray(w, np.float32) for w in (Wq, Wk, Wv))
    g1, b1l = np.asarray(ln1_g, np.float32), np.asarray(ln1_b, np.float32)
    g2, b2l = np.asarray(ln2_g, np.float32), np.asarray(ln2_b, np.float32)
    # [H,C,HS] -> [C, H*HS] with LN1 affine folded into the weights
    wq2 = Wq.transpose(1, 0, 2).reshape(C, C)
    wk2 = Wk.transpose(1, 0, 2).reshape(C, C)
    wv2 = Wv.transpose(1, 0, 2).reshape(C, C)
    bq, bk, bv = b1l @ wq2, b1l @ wk2, b1l @ wv2
    wq2, wk2, wv2 = g1[:, None] * wq2, g1[:, None] * wk2, g1[:, None] * wv2
    W1 = np.asarray(W1, np.float32)
    b1p = np.asarray(b1, np.float32) + b2l @ W1
    w1s = g2[:, None] * W1
    # DoubleRow pairing over channel-chunk pairs (2g, 2g+1):
    # packed[pp][p, g*256 + j*128 + m] = w[(2g+j)*128 + p, pp*128 + m]
    pack_qk8 = lambda w: w.reshape(NCH // 2, 2, 128, NPAIR, 128).transpose(
        3, 2, 0, 1, 4).reshape(NPAIR, 128, C)
    wqk_pk = np.stack([pack_qk8(wq2), pack_qk8(wk2)], axis=2).reshape(
        NPAIR, 128, 2048)
    wv_pk = wv2.reshape(NCH, 128, C).transpose(1, 0, 2).reshape(128, NCH * C)
    w1_pk = w1s.reshape(NCH, 128, NG, 128).transpose(2, 1, 0, 3).reshape(NG, 128, C)
    W2 = np.asarray(W2, np.float32)
    w2_b = 16.0 * W2[FP8G * 128:]
    out = {
        "wqk": f8(wqk_pk), "wv": bf(wv_pk),
        "wo": bf(Wo), "w1": bf(w1_pk), "w2b": bf(w2_b),
        "bq": f(bq), "bk": f(bk), "bv": bf(bv),
        "bo": bf(bo), "b1": f(b1p),
        "b2": f8(16.0 * np.asarray(b2, np.float32)),
    }
    if FP8G:
        w2_8 = (16.0 * W2[:FP8G * 128]).reshape(
            FP8G // 2, 2, 128, C).transpose(0, 2, 1, 3)
        out["w2"] = f8(w2_8)
    return out


def kernel(x, mask, Wq, Wk, Wv, Wo, bo, ln1_g, ln1_b, ln2_g, ln2_b, W1, b1, W2, b2):
    x = np.ascontiguousarray(np.asarray(x, dtype=np.float32))
    B = x.shape[0]
    common = _prep_inputs(x, Wq, Wk, Wv, Wo, bo, ln1_g, ln1_b,
                          ln2_g, ln2_b, W1, b1, W2, b2)
    nc = _get_program()
    in_maps = [dict(common, x=np.ascontiguousarray(x[b])) for b in range(B)]
    res = run_bass_kernel_spmd(nc, in_maps, list(range(B)))
    return np.stack([res.results[b]["y"] for b in range(B)], axis=0)


# revision 45
# speedup vs baseline: 1.8477x; 1.8477x over previous
"""Trainium2 Bass kernel for a dense transformer block.

Data-parallel over batch B=8 across 8 NeuronCores (one batch element per
core, weights replicated, no collectives).

Per core (x_b is [T=1024, C=1024] fp32):
  h  = LN1(x);  per-head q,k,v = h @ Wq/Wk/Wv;  S = q k^T / 8 with the
  "staircase" mask == block-causal at 64 granularity;  out = softmax(S) v
  x2 = x + cat(out) @ Wo + bo;  y = x2 + relu(LN2(x2) @ W1 + b1) @ W2 + b2

v15 design notes (on top of v13):
  - Q/K projections in fp8e4 DoubleRow: hT stored fp8 channel-major
    [128, NCH, T]; contraction pairs = channel-chunk pairs (2g, 2g+1) so
    hT[:, 2g:2g+2, :] is directly the DoubleRow moving AP; wq/wk
    pre-paired host-side to match.  V stays bf16 (mixed fp8xbf16 matmul)
    -- v errors pass straight through attention, q/k errors are damped
    by softmax normalization, so this is where the error budget goes.
  - attention A@V in fp8e4 DoubleRow pairing *key-block* j with j+1:
    exp(S) written fp8 into [128, 2, 1024] tiles (j-depth), V stationary
    [128, 2, 65]; ragged mask edges zero-filled (gpsimd, two-head strided
    memsets) so pairs stay legal. exp biased by -2.5 so fp8e4 never
    saturates at 240 (softmax ratio is bias-invariant).
  - attention is ACT(exp)-bound: LN normalizes moved to ACT Identity
    (per-partition scale/bias APs) in phases 1/3 where ACT idles, and the
    hT copy runs on ACT too, keeping DVE for recip/divide/bias.
  - S stays bf16 (fp8 S noise is error-prohibitive on top of qk8).
  - LN affines folded into Wq/Wk/Wv/W1 host-side; per-channel biases as
    per-partition adds on q/k, K=1 ones-row matmuls for v/bo/b2.
  - FFN2: FP8G=12 k-groups in fp8-DoubleRow (rest bf16), all 8 PSUM banks
    as full-T accumulators per column half so W2 streams exactly once.
    More fp8 (FFN1, Wo, S) busts the 2e-2 gate: HW rel err is 1.955e-2.
  - softmax denominator from the ones column of V, reciprocal + gpsimd
    partition_broadcast, divide-multiply on DVE.
  - warm-up matmuls flip the PE HAM clock gate while x streams in.
"""

import os

import numpy as np

import concourse.bass as bass
import concourse.mybir as mybir
import concourse.tile as tile
from concourse import bacc
from concourse.masks import make_identity
from concourse.bass_utils import run_bass_kernel_spmd

T, C, H, HS = 1024, 1024, 16, 64
NT = T // 128
NCH = C // 128
NPAIR = H // 2
FF = 4 * C
NG = FF // 128
FP8G = 12            # FFN2 k-groups in fp8-DoubleRow (rest bf16)
EPS = 1e-5
EXP_BIAS = -2.5      # exp(S/8 - 2.5): keeps fp8e4 exp() under 240
F32 = mybir.dt.float32
BF16 = mybir.dt.bfloat16
F8 = mybir.dt.float8e4
DR = mybir.MatmulPerfMode.DoubleRow


def _ln_stats(nc, pool, x_ap, eps_tile):
    """mean/rstd of [128,1024] fp32 tile -> (negmr, rstd) such that the
    normalize is a single ACT Copy: h = x*rstd + negmr."""
    stats = pool.tile([128, 2, 6], F32, tag="ln_stats", name="ln_stats")
    mv = pool.tile([128, 2], F32, tag="ln_mv", name="ln_mv")
    xr = x_ap.rearrange("p (s f) -> p s f", s=2)
    for s in range(2):
        nc.vector.bn_stats(out=stats[:, s, :], in_=xr[:, s, :])
    nc.vector.bn_aggr(out=mv, in_=stats)
    rstd = pool.tile([128, 1], F32, tag="ln_rstd", name="ln_rstd")
    nc.scalar.activation(
        out=rstd, in_=mv[:, 1:2],
        func=mybir.ActivationFunctionType.Sqrt,
        bias=eps_tile, scale=1.0,
    )
    nc.vector.reciprocal(out=rstd, in_=rstd)
    negmr = pool.tile([128, 1], F32, tag="ln_negmr", name="ln_negmr")
    nc.vector.scalar_tensor_tensor(
        out=negmr, in0=mv[:, 0:1], scalar=-1.0, in1=rstd,
        op0=mybir.AluOpType.mult, op1=mybir.AluOpType.mult)
    return negmr, rstd


def build_program():
    nc = bacc.Bacc("TRN2", target_bir_lowering=False, debug=False, num_devices=8)

    x_d = nc.dram_tensor("x", [T, C], F32, kind="ExternalInput").ap()
    wqk_d = nc.dram_tensor("wqk", [NPAIR, 128, 2048], F8,
                           kind="ExternalInput").ap()
    wv_d = nc.dram_tensor("wv", [128, NCH * C], BF16,
                          kind="ExternalInput").ap()   # [128, 8192] packed
    wo_d = nc.dram_tensor("wo", [C, C], BF16, kind="ExternalInput").ap()
    w1_d = nc.dram_tensor("w1", [NG, 128, C], BF16, kind="ExternalInput").ap()
    if FP8G:
        w2_d = nc.dram_tensor("w2", [FP8G // 2, 128, 2, C], F8,
                              kind="ExternalInput").ap()
    w2b_d = nc.dram_tensor("w2b", [(NG - FP8G) * 128, C], BF16,
                           kind="ExternalInput").ap()
    bq_d = nc.dram_tensor("bq", [C], F32, kind="ExternalInput").ap()
    bk_d = nc.dram_tensor("bk", [C], F32, kind="ExternalInput").ap()
    bv_d = nc.dram_tensor("bv", [C], BF16, kind="ExternalInput").ap()
    bo_d = nc.dram_tensor("bo", [C], BF16, kind="ExternalInput").ap()
    b1_d = nc.dram_tensor("b1", [FF], F32, kind="ExternalInput").ap()
    b2_d = nc.dram_tensor("b2", [C], F8, kind="ExternalInput").ap()
    y_d = nc.dram_tensor("y", [T, C], F32, kind="ExternalOutput").ap()

    reps = int(os.environ.get("KERNEL_REPS", "1"))
    with tile.TileContext(nc) as tc:
        for r in range(reps):
            _emit(nc, tc, x_d, wqk_d, wv_d, wo_d, w1_d,
                  w2_d if FP8G else None, w2b_d,
                  bq_d, bk_d, bv_d, bo_d, b1_d, b2_d, y_d,
                  warmup=(r == 0))
    nc.compile()
    return nc


def _emit(nc, tc, x_d, wqk_d, wv_d, wo_d, w1_d, w2_d, w2b_d,
          bq_d, bk_d, bv_d, bo_d, b1_d, b2_d, y_d, warmup=False):
    singles = tc.alloc_tile_pool(name="singles", bufs=1)
    # One global PSUM pool; tags: sc01 (2x2 banks), av (2x1), big (2x1).
    ps_pool = tc.alloc_tile_pool(name="ps", bufs=1, space="PSUM")
    # PE warm-up first: depends only on one DVE memset, so the HAM clock
    # gate flips to 2.4GHz while everything else (x DMA, LN1) is starting.
    if warmup:
        junk = singles.tile([128, 512], BF16, name="junk")
        nc.vector.memset(junk, 0.0)
        for _ in range(48):
            ps = ps_pool.tile(
                [128, 512], F32, tag="av", bufs=2, name="ps_warm")
            nc.tensor.matmul(ps, junk[:, 0:128], junk, start=True, stop=True)
    identf = singles.tile([128, 128], F32, name="identf")
    make_identity(nc, identf)
    ident = singles.tile([128, 128], BF16, name="ident")
    nc.vector.tensor_copy(out=ident, in_=identf)
    eps_tile = singles.tile([128, 1], F32, name="eps")
    nc.vector.memset(eps_tile, EPS)
    ebias_tile = singles.tile([128, 1], F32, name="ebias")
    nc.vector.memset(ebias_tile, EXP_BIAS)
    ones_r = singles.tile([1, 128], BF16, name="ones_r")
    nc.vector.memset(ones_r, 1.0)
    ones_f8 = singles.tile([1, 128], F8, name="ones_f8")
    nc.vector.memset(ones_f8, 1.0)
    # bias tiles are allocated here, but their DMA issues are deferred
    # until after the first x/wv loads: the SP engine serializes DMA
    # issues (~0.8us each) and LN1's x tiles are on the critical path.
    b1_sb = singles.tile([128, NG], F32, name="b1_sb")
    bq_sb = singles.tile([128, NPAIR], F32, name="bq_sb")
    bk_sb = singles.tile([128, NPAIR], F32, name="bk_sb")
    bv_sb = singles.tile([1, C], BF16, name="bv_sb")
    bo_sb = singles.tile([1, C], BF16, name="bo_sb")
    b2_sb = singles.tile([1, C], F8, name="b2_sb")

    def load_biases():
        nc.sync.dma_start(out=bv_sb, in_=bv_d.unsqueeze(0))
        nc.sync.dma_start(out=bq_sb,
                          in_=bq_d.rearrange("(g p) -> p g", p=128))
        nc.sync.dma_start(out=bk_sb,
                          in_=bk_d.rearrange("(g p) -> p g", p=128))
        nc.sync.dma_start(out=bo_sb, in_=bo_d.unsqueeze(0))
        nc.sync.dma_start(out=b1_sb,
                          in_=b1_d.rearrange("(g p) -> p g", p=128))
        nc.sync.dma_start(out=b2_sb, in_=b2_d.unsqueeze(0))

    ln_pool = tc.alloc_tile_pool(name="ln", bufs=3)

    def big():
        return ps_pool.tile([128, 512], F32, tag="big", bufs=2, name="ps_big")

    hT_pool = tc.alloc_tile_pool(name="hTp", bufs=1)
    hT = hT_pool.tile([128, NCH, T], F8, name="hT")
    x2_pool = tc.alloc_tile_pool(name="x2p", bufs=1)
    x2 = x2_pool.tile([128, NT, C], F32, name="x2")
    w1_pool = tc.alloc_tile_pool(name="w1p", bufs=4)
    wo_pool = tc.alloc_tile_pool(name="wop", bufs=1)
    wo_t = wo_pool.tile([128, NCH, C], BF16, name="wo_t")
    w_pool = tc.alloc_tile_pool(name="wqk", bufs=3)
    v_pool = tc.alloc_tile_pool(name="vAp", bufs=1)
    v_all = v_pool.tile([128, NT, H * 65], F8, name="v_all")
    for hh in range(H):
        nc.gpsimd.memset(v_all[:, :, 65 * hh + 64:65 * hh + 65], 1.0)

    h2T_pool = tc.alloc_tile_pool(name="h2Tp", bufs=1, side="right")
    h2T = h2T_pool.tile([128, NCH, T], BF16, name="h2T")
    outT_pool = tc.alloc_tile_pool(name="outTp", bufs=1, side="right")
    outT = outT_pool.tile([128, NPAIR, T], BF16, name="outT")

    wqk_tiles = {}

    def load_pair(pp):
        wqk_t = w_pool.tile([128, 2048], F8, tag="wqk", name="wqk_t")
        nc.sync.dma_start(out=wqk_t, in_=wqk_d[pp])
        wqk_tiles[pp] = wqk_t

    # ---- Phase 1: LN1 -> hT (channel-major, fp8) + V (fp8 DoubleRow) ----
    v_view = v_all.rearrange("p i (h d) -> p i h d", h=H)
    with tc.tile_pool(name="h", bufs=3) as h_pool, \
         tc.tile_pool(name="xin1", bufs=4) as x_pool, \
         tc.tile_pool(name="wvg", bufs=1) as wv_pool:
        wv_t = wv_pool.tile([128, NCH * 1024], BF16, name="wv_t")

        def load_x(ii):
            x_t = x_pool.tile([128, C], F32, tag="x", name="x_t")
            nc.sync.dma_start(
                out=x_t, in_=x_d[ii * 128:(ii + 1) * 128, :])
            return x_t

        x_ts = {0: load_x(0), 1: load_x(1)}
        for grp in range(2):
            nc.sync.dma_start(
                out=wv_t[:, grp * 4096:(grp + 1) * 4096],
                in_=wv_d[:, grp * 4096:(grp + 1) * 4096])
        load_biases()
        load_pair(0)
        load_pair(1)
        stats = {0: _ln_stats(nc, ln_pool, x_ts[0], eps_tile)}
        for i in range(NT):
            if i + 2 < NT:
                x_ts[i + 2] = load_x(i + 2)
            negmr, rstd = stats.pop(i)
            x_t = x_ts.pop(i)
            h_t = h_pool.tile([128, C], BF16, tag="h", name="h_t")
            nc.scalar.activation(
                out=h_t, in_=x_t,
                func=mybir.ActivationFunctionType.Identity,
                scale=rstd, bias=negmr)
            ps_tr = ps_pool.tile([128, 1024], BF16, tag="av",
                                 bufs=2, name="ps_tr")
            for j in range(NCH):
                nc.tensor.transpose(
                    ps_tr[:, j * 128:(j + 1) * 128],
                    h_t[:, j * 128:(j + 1) * 128], ident)
            nc.scalar.activation(
                out=hT[:, :, i * 128:(i + 1) * 128],
                in_=ps_tr.rearrange("p (j t) -> p j t", j=NCH),
                func=mybir.ActivationFunctionType.Copy)
            if i + 1 < NT:
                stats[i + 1] = _ln_stats(nc, ln_pool, x_ts[i + 1], eps_tile)
            for grp in range(2):
                ps_v = big()
                nc.tensor.matmul(
                    ps_v, ones_r, bv_sb[0:1, grp * 512:(grp + 1) * 512],
                    start=True, stop=False, skip_group_check=True)
                for j in range(NCH):
                    nc.tensor.matmul(
                        ps_v, hT[:, j, i * 128:(i + 1) * 128],
                        wv_t[:, j * 1024 + grp * 512:j * 1024 + (grp + 1) * 512],
                        skip_group_check=True,
                        start=False, stop=(j == NCH - 1))
                nc.vector.tensor_copy(
                    out=v_view[:, i, grp * 8:(grp + 1) * 8, 0:64],
                    in_=ps_v.rearrange("p (h d) -> p h d", h=8))

    # hoist the wo load (one 3D-AP DMA issue): SP is idle during attention
    nc.sync.dma_start(
        out=wo_t, in_=wo_d.rearrange("(ch p) c -> p ch c", p=128))

    # ---- Phase 2: per head-pair QK + attention (V stationary) ----
    with tc.tile_pool(name="qk", bufs=2) as qk_pool, \
         tc.tile_pool(name="expS", bufs=14) as e_pool, \
         tc.tile_pool(name="rec", bufs=4) as r_pool, \
         tc.tile_pool(name="rbc", bufs=4) as rb_pool:
        def qkproj(p):
            wqk_t = wqk_tiles.pop(p)
            qT = qk_pool.tile([128, T], BF16, tag="qT", name="qT")
            kT = qk_pool.tile([128, T], BF16, tag="kT", name="kT")
            for di, (dst, bias) in enumerate(((qT, bq_sb), (kT, bk_sb))):
                for half in range(2):
                    ps = big()
                    for g in range(NCH // 2):
                        nc.tensor.matmul(
                            ps,
                            wqk_t[:, di * 1024 + g * 256:
                                  di * 1024 + (g + 1) * 256].rearrange(
                                      "p (j m) -> p j m", j=2),
                            hT[:, 2 * g:2 * g + 2,
                               half * 512:(half + 1) * 512],
                            perf_mode=DR,
                            start=(g == 0), stop=(g == NCH // 2 - 1))
                    nc.vector.tensor_scalar(
                        out=dst[:, half * 512:(half + 1) * 512], in0=ps,
                        scalar1=bias[:, p:p + 1], scalar2=None,
                        op0=mybir.AluOpType.add)
            return qT, kT

        def emit_s(qT, kT, th):
            t0 = th * 512
            njt = (th + 1) * 4
            eT = [None] * (njt // 2)
            for jp in range(njt // 2):
                et = e_pool.tile([128, 2, 1024], F8, tag="e", name="eS_t")
                c0e = max(0, (2 * jp) * 128 - t0)
                c0o = max(0, (2 * jp + 1) * 128 - t0)
                if c0o > c0e:
                    # odd block masked where even block isn't: zero so
                    # the DoubleRow pair contributes nothing there
                    # (both heads in one strided memset, on gpsimd)
                    nc.gpsimd.memset(
                        et[:, 1, :].rearrange("p (h q) -> p h q", h=2)
                        [:, :, c0e:c0o], 0.0)
                for dj in range(2):
                    j = 2 * jp + dj
                    c0 = max(0, j * 128 - t0)
                    ps = ps_pool.tile([128, 1024], F32, tag="s2",
                                      bufs=2, name="ps_sc")
                    for hh in range(2):
                        hsl = slice(hh * 64, (hh + 1) * 64)
                        nc.tensor.matmul(
                            ps[:, hh * 512 + c0:(hh + 1) * 512],
                            kT[hsl, j * 128:(j + 1) * 128],
                            qT[hsl, t0 + c0:t0 + 512],
                            start=True, stop=True,
                            tile_position=(hh * 64, 0))
                    pv = ps.rearrange("p (h q) -> p h q", h=2)
                    ev = et[:, dj, :].rearrange("p (h q) -> p h q", h=2)
                    nc.scalar.activation(
                        out=ev[:, :, c0:512], in_=pv[:, :, c0:512],
                        func=mybir.ActivationFunctionType.Exp,
                        scale=float(HS) ** -0.5, bias=ebias_tile)
                    if j * 128 >= t0:
                        nc.gpsimd.memset(
                            et[64:128, dj, :].rearrange(
                                "p (h q) -> p h q", h=2)
                            [:, :, c0:c0 + 64], 0.0)
                eT[jp] = et
            return eT

        def emit_av(p, th, eT):
            t0 = th * 512
            njt = (th + 1) * 4
            for hh in range(2):
                head = 2 * p + hh
                ps_av = ps_pool.tile([128, 512], F32, tag="av", bufs=2,
                                     name="ps_av")
                for jp in range(njt // 2):
                    c0 = max(0, (2 * jp) * 128 - t0)
                    nc.tensor.matmul(
                        ps_av[0:65, c0:512],
                        v_all[:, 2 * jp:2 * jp + 2,
                              65 * head:65 * head + 65],
                        eT[jp][:, :, hh * 512 + c0:(hh + 1) * 512],
                        perf_mode=DR,
                        start=(jp == 0), stop=(jp == njt // 2 - 1))
                rc = r_pool.tile([128, 512], F32, tag="rc", name="rc")
                nc.vector.reciprocal(out=rc[0:1, :], in_=ps_av[64:65, :])
                rb = rb_pool.tile([128, 512], F32, tag="rb", name="rb")
                nc.gpsimd.partition_broadcast(rb[0:64, :], rc[0:1, :])
                nc.vector.tensor_tensor(
                    out=outT[64 * hh:64 * (hh + 1), p, t0:t0 + 512],
                    in0=ps_av[0:64, :], in1=rb[0:64, :],
                    op=mybir.AluOpType.mult)

        # software pipeline: AV of pair p-1 sits between S(th0) and S(th1)
        # of pair p, and pair p+1's projections follow S(th1), so the ACT
        # exp() queue never starves the PE (and vice versa).
        qk_cur = qkproj(0)
        pend = None
        for p in range(NPAIR):
            if p + 2 < NPAIR:
                load_pair(p + 2)
            qT, kT = qk_cur
            eTs = {0: emit_s(qT, kT, 0)}
            if pend is not None:
                for th in range(2):
                    emit_av(pend[0], th, pend[1][th])
            eTs[1] = emit_s(qT, kT, 1)
            if p + 1 < NPAIR:
                qk_cur = qkproj(p + 1)
            pend = (p, eTs)
        for th in range(2):
            emit_av(pend[0], th, pend[1][th])
    v_pool.release()
    w_pool.release()

    # hoist the first FFN1 weight tiles
    w1_tiles = {}

    def load_w1(gg):
        w1_t = w1_pool.tile([128, C], BF16, tag="w1", name="w1_t")
        nc.sync.dma_start(out=w1_t, in_=w1_d[gg])
        w1_tiles[gg] = w1_t

    for gg in range(3):
        load_w1(gg)

    # ---- Phase 3+4: projection + residual (+bo) + LN2 -> h2T ----
    # Skewed: transposes of tile i-1 are emitted after tile i's proj
    # matmuls so the PE never waits on the DVE/ACT LN2 chain.
    with tc.tile_pool(name="xin2", bufs=4) as x_pool:

        def load_x2(ii):
            x_t = x_pool.tile([128, C], F32, tag="x", name="x_t2")
            nc.sync.dma_start(
                out=x_t, in_=x_d[ii * 128:(ii + 1) * 128, :])
            return x_t

        x_ts2 = {0: load_x2(0), 1: load_x2(1)}
        h_ts = {}

        def emit_tr2(ii):
            h_t = h_ts.pop(ii)
            ps_tr = ps_pool.tile([128, 1024], BF16, tag="s2",
                                 bufs=2, name="ps_tr2")
            for j in range(NCH):
                nc.tensor.transpose(
                    ps_tr[:, j * 128:(j + 1) * 128],
                    h_t[:, j * 128:(j + 1) * 128], ident)
            nc.vector.tensor_copy(
                out=h2T[:, :, ii * 128:(ii + 1) * 128],
                in_=ps_tr.rearrange("p (j t) -> p j t", j=NCH))

        for i in range(NT):
            if i + 2 < NT:
                x_ts2[i + 2] = load_x2(i + 2)
            x_t = x_ts2.pop(i)
            for half in range(2):
                psh = ps_pool.tile([128, 512], F32, tag="av", bufs=2,
                                   name="ps_pr")
                nc.tensor.matmul(
                    psh, ones_r, bo_sb[0:1, half * 512:(half + 1) * 512],
                    start=True, stop=False)
                for ch in range(NCH):
                    nc.tensor.matmul(
                        psh, outT[:, ch, i * 128:(i + 1) * 128],
                        wo_t[:, ch, half * 512:(half + 1) * 512],
                        start=False, stop=(ch == NCH - 1))
                nc.vector.tensor_add(
                    out=x2[:, i, half * 512:(half + 1) * 512],
                    in0=psh, in1=x_t[:, half * 512:(half + 1) * 512])
            negmr, rstd = _ln_stats(nc, ln_pool, x2[:, i, :], eps_tile)
            h_t = x_pool.tile([128, C], BF16, tag="h2", name="h2_t")
            nc.scalar.activation(
                out=h_t, in_=x2[:, i, :],
                func=mybir.ActivationFunctionType.Identity,
                scale=rstd, bias=negmr)
            h_ts[i] = h_t
            if i >= 1:
                emit_tr2(i - 1)
        emit_tr2(NT - 1)
    outT_pool.release()
    wo_pool.release()

    # ---- Phase 5: FFN. W1 streamed once into full-T uT; W2 in 2 passes ----
    with tc.tile_pool(name="w2", bufs=8) as w2_pool, \
         tc.tile_pool(name="uTp", bufs=1) as uT_pool, \
         tc.tile_pool(name="yout", bufs=4) as out_pool:
        if FP8G:
            uT8 = uT_pool.tile([128, FP8G, T], F8, name="uT8")
        uTb = uT_pool.tile([128, NG - FP8G, T], BF16, name="uTb")
        for g in range(NG):
            if g + 3 < NG:
                load_w1(g + 3)
            w1_t = w1_tiles.pop(g)
            for th in range(2):
                ps = big()
                for j in range(NCH):
                    nc.tensor.matmul(
                        ps, w1_t[:, j * 128:(j + 1) * 128],
                        h2T[:, j, th * 512:(th + 1) * 512],
                        start=(j == 0), stop=(j == NCH - 1))
                udst = (uT8[:, g, :] if g < FP8G
                        else uTb[:, g - FP8G, :])
                nc.vector.tensor_scalar(
                    out=udst[:, th * 512:(th + 1) * 512],
                    in0=ps,
                    scalar1=b1_sb[:, g:g + 1], scalar2=0.0,
                    op0=mybir.AluOpType.add, op1=mybir.AluOpType.max)
        # FFN2: all 8 PSUM banks as full-T accumulators per column half,
        # so W2 streams exactly once.
        for chh in range(2):
            hsl = slice(chh * 512, (chh + 1) * 512)
            ps_w = [ps_pool.tile([128, 1024], F32, tag="s2",
                                 bufs=2, name=f"ps_w{iw}")
                    for iw in range(2)]
            ps_f = [ps_w[iw // 2][:, (iw % 2) * 512:(iw % 2 + 1) * 512]
                    for iw in range(4)]
            ps_f += [ps_pool.tile([128, 512], F32, tag="av", bufs=2,
                                  name=f"ps_a{iw}") for iw in range(2)]
            ps_f += [big(), big()]
            for it in range(8):
                nc.tensor.matmul(
                    ps_f[it], ones_f8, b2_sb[0:1, hsl],
                    start=True, stop=False, skip_group_check=True)
            for k2 in range(FP8G // 2):
                w2_t = w2_pool.tile([128, 2, 512], F8,
                                    tag="w28", name="w2_t8")
                nc.sync.dma_start(out=w2_t, in_=w2_d[k2][:, :, hsl])
                for it in range(8):
                    nc.tensor.matmul(
                        ps_f[it],
                        uT8[:, 2 * k2:2 * k2 + 2, it * 128:(it + 1) * 128],
                        w2_t,
                        perf_mode=DR, skip_group_check=True,
                        start=False, stop=False)
            for kb in range(NG - FP8G):
                w2_t = w2_pool.tile([128, 512], BF16, tag="w2b", name="w2_tb")
                nc.sync.dma_start(
                    out=w2_t, in_=w2b_d[kb * 128:(kb + 1) * 128, hsl])
                for it in range(8):
                    nc.tensor.matmul(
                        ps_f[it],
                        uTb[:, kb, it * 128:(it + 1) * 128],
                        w2_t, skip_group_check=True,
                        start=False, stop=(kb == NG - FP8G - 1))
            for it in range(8):
                o_t = out_pool.tile([128, 512], F32, tag="y", name="y_t")
                nc.vector.scalar_tensor_tensor(
                    out=o_t, in0=ps_f[it], scalar=1.0 / 16.0,
                    in1=x2[:, it, hsl],
                    op0=mybir.AluOpType.mult, op1=mybir.AluOpType.add)
                nc.sync.dma_start(
                    out=y_d[it * 128:(it + 1) * 128, hsl], in_=o_t)
    h2T_pool.release()
    w1_pool.release()
    x2_pool.release()
    hT_pool.release()
    ps_pool.release()
    ln_pool.release()
    singles.release()


_NC_CACHE = {}


def _get_program():
    if "nc" not in _NC_CACHE:
        _NC_CACHE["nc"] = build_program()
    return _NC_CACHE["nc"]


def _prep_inputs(x, Wq, Wk, Wv, Wo, bo, ln1_g, ln1_b, ln2_g, ln2_b, W1, b1, W2, b2):
    import ml_dtypes
    BF = ml_dtypes.bfloat16
    F8np = ml_dtypes.float8_e4m3
    f = lambda a: np.ascontiguousarray(np.asarray(a, dtype=np.float32))
    bf = lambda a: np.ascontiguousarray(np.asarray(a, np.float32).astype(BF))
    f8 = lambda a: np.ascontiguousarray(np.asarray(a, np.float32).astype(F8np))
    Wq, Wk, Wv = (np.asarray(w, np.float32) for w in (Wq, Wk, Wv))
    g1, b1l = np.asarray(ln1_g, np.float32), np.asarray(ln1_b, np.float32)
    g2, b2l = np.asarray(ln2_g, np.float32), np.asarray(ln2_b, np.float32)
    # [H,C,HS] -> [C, H*HS] with LN1 affine folded into the weights
    wq2 = Wq.transpose(1, 0, 2).reshape(C, C)
    wk2 = Wk.transpose(1, 0, 2).reshape(C, C)
    wv2 = Wv.transpose(1, 0, 2).reshape(C, C)
    bq, bk, bv = b1l @ wq2, b1l @ wk2, b1l @ wv2
    wq2, wk2, wv2 = g1[:, None] * wq2, g1[:, None] * wk2, g1[:, None] * wv2
    W1 = np.asarray(W1, np.float32)
    b1p = np.asarray(b1, np.float32) + b2l @ W1
    w1s = g2[:, None] * W1
    # DoubleRow pairing over channel-chunk pairs (2g, 2g+1):
    # packed[pp][p, g*256 + j*128 + m] = w[(2g+j)*128 + p, pp*128 + m]
    pack_qk8 = lambda w: w.reshape(NCH // 2, 2, 128, NPAIR, 128).transpose(
        3, 2, 0, 1, 4).reshape(NPAIR, 128, C)
    wqk_pk = np.stack([pack_qk8(wq2), pack_qk8(wk2)], axis=2).reshape(
        NPAIR, 128, 2048)
    wv_pk = wv2.reshape(NCH, 128, C).transpose(1, 0, 2).reshape(128, NCH * C)
    w1_pk = w1s.reshape(NCH, 128, NG, 128).transpose(2, 1, 0, 3).reshape(NG, 128, C)
    W2 = np.asarray(W2, np.float32)
    w2_b = 16.0 * W2[FP8G * 128:]
    out = {
        "wqk": f8(wqk_pk), "wv": bf(wv_pk),
        "wo": bf(Wo), "w1": bf(w1_pk), "w2b": bf(w2_b),
        "bq": f(bq), "bk": f(bk), "bv": bf(bv),
        "bo": bf(bo), "b1": f(b1p),
        "b2": f8(16.0 * np.asarray(b2, np.float32)),
    }
    if FP8G:
        w2_8 = (16.0 * W2[:FP8G * 128]).reshape(
            FP8G // 2, 2, 128, C).transpose(0, 2, 1, 3)
        out["w2"] = f8(w2_8)
    return out


def kernel(x, mask, Wq, Wk, Wv, Wo, bo, ln1_g, ln1_b, ln2_g, ln2_b, W1, b1, W2, b2):
    x = np.ascontiguousarray(np.asarray(x, dtype=np.float32))
    B = x.shape[0]
    common = _prep_inputs(x, Wq, Wk, Wv, Wo, bo, ln1_g, ln1_b,
                          ln2_g, ln2_b, W1, b1, W2, b2)
    nc = _get_program()
    in_maps = [dict(common, x=np.ascontiguousarray(x[b])) for b in range(B)]
    res = run_bass_kernel_spmd(nc, in_maps, list(range(B)))
    return np.stack([res.results[b]["y"] for b in range(B)], axis=0)
